# revision 26
# baseline (speedup 1.0000x reference)
"""Block-circulant matmul kernel for 8 Trainium2 NeuronCores.

Reference op (per token row x of shape (4096,)):
    y = (x*d) @ M + bias,  M[(j,m),(i,n)] = W[i,j,(m-n)%256]  (circulant blocks)

Real-DFT factorization in three matmul stages per core, data-parallel over
the batch (1024 tokens/core), all bf16 with fp32 PSUM accumulation:
  stage1: per input block j, project onto the 256-col real DFT basis
  stage2: per frequency-group G, one 128x128 block-diag mix (+bias fold)
  stage3: per output block i, inverse real DFT basis
Two SBUF->SBUF partition-shuffle DMA passes (frequency-major <->
block-major) sit between the stages; their 8-partition sides are placed
stride-4 across SBUF ports so each transfer spreads over 8 DMA engines.

Phase-ordered emission with token-halves h0/h1 for stage1/stage2 overlap:
  A: x(h0) loads + stage1(h0)
  B: x(h1) loads + stage1(h1) + shuffle1(h0)
  C: shuffle1(h1) + stage2 + shuffle2
  D: stage3 + output stores
The ACT (scalar) sequencer is kept almost free of DMA issue duty so its
cycles go to PSUM evacuation; sync carries shuffle1 + x(h0) + stores,
gpsimd carries x(h1) + half of shuffle2.

Self-contained: shapes hardcoded; no sibling imports.
"""
import os
import sys

for _p in ("/root/.axon_site", "/root/.axon_site/_ro/trn_rl_repo", "/root/.axon_site/_ro/pypackages"):
    if _p not in sys.path:
        sys.path.append(_p)

import numpy as np
import ml_dtypes

import concourse.bass as bass
import concourse.tile as tile
from concourse import bacc, mybir
from concourse import bass_utils

N_CORES = 8
B = 8192
D = 4096
BS = 256
K = 16             # blocks per side
NSLOT = BS // 2    # 128 frequency pair-slots
NT = B // N_CORES  # tokens per core (1024)
TC = 512           # token chunk
NCH = NT // TC     # chunks (2)

F32 = mybir.dt.float32
F32R = mybir.dt.float32r
BF16 = mybir.dt.bfloat16
BF16_NP = ml_dtypes.bfloat16

LAST_EXEC_NS = None
_CACHE = {}


# ---------------------------------------------------------------- host math

def _canonical_mats(W):
    m = np.arange(BS)
    T = np.zeros((BS, BS), np.float64)
    T[:, 0] = 1.0
    T[:, 1] = (-1.0) ** m
    for f in range(1, NSLOT):
        T[:, 2 * f] = np.cos(2 * np.pi * f * m / BS)
        T[:, 2 * f + 1] = np.sin(2 * np.pi * f * m / BS)

    Wf = np.fft.fft(W.astype(np.float64), axis=-1)
    p = Wf.real
    q = -Wf.imag

    jj = np.arange(K)
    M_slot = np.zeros((NSLOT, 2 * K, 2 * K), np.float64)
    for f in range(1, NSLOT):
        pf, qf = p[:, :, f], q[:, :, f]          # [i, j]
        M_slot[f][np.ix_(2 * jj, 2 * jj)] = pf.T
        M_slot[f][np.ix_(2 * jj + 1, 2 * jj)] = qf.T
        M_slot[f][np.ix_(2 * jj, 2 * jj + 1)] = qf.T
        M_slot[f][np.ix_(2 * jj + 1, 2 * jj + 1)] = -pf.T
    M_slot[0][np.ix_(2 * jj, 2 * jj)] = p[:, :, 0].T
    M_slot[0][np.ix_(2 * jj + 1, 2 * jj + 1)] = p[:, :, NSLOT].T

    n = np.arange(BS)
    R = np.zeros((BS, BS), np.float64)
    R[0, :] = 1.0 / BS
    R[1, :] = ((-1.0) ** n) / BS
    for f in range(1, NSLOT):
        R[2 * f, :] = 2.0 / BS * np.cos(2 * np.pi * f * n / BS)
        R[2 * f + 1, :] = -2.0 / BS * np.sin(2 * np.pi * f * n / BS)
    return T, M_slot, R


def _fft_host_mats(W, bias):
    T, M_slot, R = _canonical_mats(W)
    p_idx = np.arange(128)

    # tb_dram (128, 4*128): [p, (mt*2+pb)*128+col] = T[mt*128+p, colmap(pb,col)]
    # u_sb partition p holds comp (gl, qc) with p = 32*(gl%4) + 4*qc + gl//4
    # so each gl-group's 8 partitions spread stride-4 across SBUF ports
    tb = np.zeros((128, 512), np.float32)
    for pb in range(2):
        gl_u = 4 * (p_idx % 4) + p_idx // 32
        qc_u = (p_idx // 4) % 8
        slot = 64 * pb + 4 * gl_u + qc_u // 2
        c = qc_u % 2
        cols = 2 * slot + c
        for mt in range(2):
            tb[:, (mt * 2 + pb) * 128:(mt * 2 + pb + 1) * 128] = \
                T[mt * 128:(mt + 1) * 128, :][:, cols]

    # mix_dram (128, 32*128) bf16: [row, G*128+col]
    mix = np.zeros((128, 32 * 128), np.float64)
    kk = np.arange(K)
    for G in range(32):
        MG = np.zeros((128, 128), np.float64)
        for r in range(4):
            blk = M_slot[4 * G + r]
            for c in range(2):
                for cp in range(2):
                    MG[np.ix_(16 * (2 * r + c) + kk, 16 * (2 * r + cp) + kk)] = \
                        blk[np.ix_(2 * kk + c, 2 * kk + cp)]
        mix[:, G * 128:(G + 1) * 128] = MG

    # r_dram (128, 4*128): v_sb partition p holds comp (gl, q2) with
    # p = 32*(gl%4) + 4*q2 + gl//4 (same stride-4 port spread)
    rd = np.zeros((128, 512), np.float64)
    for kt in range(2):
        gl = 4 * (p_idx % 4) + p_idx // 32
        q = ((p_idx // 4) % 8) // 2
        c = (p_idx // 4) % 2
        rows = 2 * (64 * kt + 4 * gl + q) + c
        for nb in range(2):
            rd[:, (kt * 2 + nb) * 128:(kt * 2 + nb + 1) * 128] = \
                R[rows, :][:, nb * 128:(nb + 1) * 128]

    # beta: per output block i solve R^T beta_i = bias_i; fold into stage-2
    # layout (128, 32) f32: [16*qc + i, G] = beta_i[2*(4G+q)+c], qc = 2q+c
    beta = np.zeros((128, 32), np.float64)
    RTinv = np.linalg.inv(R.T)
    for i in range(K):
        bi = RTinv @ bias[i * BS:(i + 1) * BS].astype(np.float64)
        for G in range(32):
            for q in range(4):
                for c in range(2):
                    qc = 2 * q + c
                    beta[16 * qc + i, G] = bi[2 * (4 * G + q) + c]
    return (tb.astype(BF16_NP),
            mix.astype(BF16_NP),
            rd.astype(BF16_NP),
            beta.astype(np.float32))


# ---------------------------------------------------------------- fft kernel

def _build_fft_nc():
    nc = bacc.Bacc("TRN2", target_bir_lowering=False, debug=False)
    # x_dev: row j*128+p, col h*1024 + mt*512 + t  (2KB contiguous lines)
    xT = nc.dram_tensor("xT", [K * 128, 2 * NT], BF16, kind="ExternalInput").ap()
    tb_d = nc.dram_tensor("tb", [128, 512], BF16, kind="ExternalInput").ap()
    mix_d = nc.dram_tensor("mix", [128, 32 * 128], BF16, kind="ExternalInput").ap()
    r_d = nc.dram_tensor("rmat", [128, 512], BF16, kind="ExternalInput").ap()
    beta_d = nc.dram_tensor("beta", [128, 32], F32, kind="ExternalInput").ap()
    yT = nc.dram_tensor("yT", [D, NT], BF16, kind="ExternalOutput").ap()

    ec = [0]

    def evac(dst, src):
        # alternate PSUM->SBUF evacuation between DVE and ACT
        if ec[0] % 2 == 0:
            nc.vector.tensor_copy(dst, src)
        else:
            nc.scalar.copy(dst, src)
        ec[0] += 1

    def evac_add(dst, src, beta_col):
        if ec[0] % 2 == 0:
            nc.vector.tensor_scalar_add(dst, src, beta_col)
        else:
            nc.scalar.add(dst, src, beta_col)
        ec[0] += 1

    with tile.TileContext(nc) as tc:
        with (
            tc.tile_pool(name="consts", bufs=1) as consts,
            tc.tile_pool(name="xpool", bufs=6) as xpool,
            tc.tile_pool(name="upool", bufs=2) as upool,
            tc.tile_pool(name="u2pool", bufs=30) as u2pool,
            tc.tile_pool(name="v2pool", bufs=5) as v2pool,
            tc.tile_pool(name="vpool", bufs=1) as vpool,
            tc.tile_pool(name="ypool", bufs=2) as ypool,
            tc.tile_pool(name="psBig", bufs=4, space="PSUM") as psBig,
        ):
            tb_sb = consts.tile([128, 512], BF16)
            nc.gpsimd.dma_start(tb_sb[:], tb_d[:])
            mix_sb = consts.tile([128, 32 * 128], BF16)
            nc.gpsimd.dma_start(mix_sb[:], mix_d[:])
            r_sb = consts.tile([128, 512], BF16)
            nc.gpsimd.dma_start(r_sb[:], r_d[:])
            beta_sb = consts.tile([128, 32], F32)
            nc.gpsimd.dma_start(beta_sb[:], beta_d[:])

            # u_sb[pb]: col = j*NT + h*512 + t
            u_sb = []
            for pb in range(2):
                u_pb = upool.tile([128, K * NT], BF16, tag="u")
                u_sb.append(u_pb)
            # v_sb: col = kt*16*NT + i*NT + t
            v_sb = vpool.tile([128, 32 * NT], BF16, tag="v")

            xt = {}

            def load_x(j, h):
                t = xpool.tile([128, NT], BF16, tag="x", name="xt")
                eng = nc.sync if h == 0 else nc.gpsimd
                eng.dma_start(t[:], xT[j * 128:(j + 1) * 128, h * NT:(h + 1) * NT])
                xt[(j, h)] = t

            def s1_pair(h, j0):
                # two j-blocks share each stationary load (runs of 2)
                ps = {}
                for j in (j0, j0 + 1):
                    ps[j] = psBig.tile([128, 1024], F32, tag="psb", name="ps1")
                for pb in range(2):
                    for mt in range(2):
                        for j in (j0, j0 + 1):
                            nc.tensor.matmul(
                                ps[j][:, pb * 512:pb * 512 + 512],
                                tb_sb[:, (mt * 2 + pb) * 128:(mt * 2 + pb + 1) * 128],
                                xt[(j, h)][:, mt * 512:(mt + 1) * 512],
                                start=(mt == 0), stop=(mt == 1),
                            )
                for j in (j0, j0 + 1):
                    xt.pop((j, h))
                    evac(u_sb[0][:, j * NT + h * 512:j * NT + h * 512 + 512],
                         ps[j][:, 0:512])
                    evac(u_sb[1][:, j * NT + h * 512:j * NT + h * 512 + 512],
                         ps[j][:, 512:1024])

            u2t = {}

            def issue_sh1(G, h, eng=None):
                t = u2pool.tile([128, 512], BF16, tag="u2", name="u2t")
                pb, gl = G // 16, G % 16
                st = 32 * (gl % 4) + gl // 4
                src = (u_sb[pb][st:st + 29:4, :]
                       .rearrange("p (j g t) -> p j g t", j=16, g=2)
                       [:, :, h, :])
                if eng is None:
                    eng = nc.sync
                eng.dma_start(t[:], src)
                u2t[(G, h)] = t

            # ---- phase A: x(h0) loads + stage1(h0) ----
            for j in range(K):
                load_x(j, 0)
                if j % 2 == 1:
                    s1_pair(0, j - 1)

            # ---- phase B: x(h1) loads + stage1(h1) + sh1(h0) ----
            for jp in range(8):
                if jp == 0:
                    for q in range(6):
                        load_x(q, 1)
                else:
                    for q in (2 * jp + 4, 2 * jp + 5):
                        if q < K:
                            load_x(q, 1)
                j = 2 * jp + 1
                issue_sh1(j - 1, 0)
                issue_sh1(j, 0)
                issue_sh1(16 + j - 1, 0)
                issue_sh1(16 + j, 0)
                s1_pair(1, j - 1)

            # ---- phase C: sh1(h1) + stage2 + sh2 ----
            SH1_AHEAD = 12
            for G in range(SH1_AHEAD):
                # first wave rides the empty ACT ring so phase C starts
                # without waiting for sync's sh1(h0) backlog to drain
                issue_sh1(G, 1, eng=(nc.scalar if G < 6 else None))
            for G in range(32):
                if G + SH1_AHEAD < 32:
                    issue_sh1(G + SH1_AHEAD, 1)
                pb, gl = G // 16, G % 16
                v2_t = v2pool.tile([128, NT], BF16, tag="v2")
                ps2 = psBig.tile([128, 1024], F32, tag="psb", name="ps2")
                for h in range(2):
                    nc.tensor.matmul(
                        ps2[:, h * 512:(h + 1) * 512],
                        mix_sb[:, G * 128:(G + 1) * 128],
                        u2t.pop((G, h))[:],
                        start=True, stop=True,
                    )
                evac_add(v2_t[:], ps2[:], beta_sb[:, G:G + 1])
                st = 32 * (gl % 4) + gl // 4
                kt = pb
                eng = nc.gpsimd if G % 2 == 0 else nc.sync
                eng.dma_start(
                    v_sb[st:st + 29:4, kt * 16 * NT:(kt + 1) * 16 * NT],
                    v2_t[:],
                )

            # ---- phase D: stage3 + output stores ----
            for og in range(8):                   # output groups of 4 ob
                y_t = ypool.tile([128, 4 * NT], BF16, tag="y")
                for ow in range(4):
                    ob = og * 4 + ow
                    i, nb = ob // 2, ob % 2
                    ps3 = psBig.tile([128, 1024], F32, tag="psb", name="ps3")
                    for kt in range(2):
                        for th in range(NCH):
                            nc.tensor.matmul(
                                ps3[:, th * TC:(th + 1) * TC],
                                r_sb[:, (kt * 2 + nb) * 128:(kt * 2 + nb + 1) * 128],
                                v_sb[:, (kt * 16 + i) * NT + th * TC:
                                     (kt * 16 + i) * NT + (th + 1) * TC],
                                start=(kt == 0), stop=(kt == 1),
                            )
                    evac(y_t[:, ow * NT:(ow + 1) * NT], ps3[:])
                nc.sync.dma_start(
                    yT[og * 4 * 128:(og + 1) * 4 * 128, :].rearrange(
                        "(o p) t -> p o t", p=128),
                    y_t[:].rearrange("p (o t) -> p o t", o=4),
                )
    nc.compile()
    return nc


# ---------------------------------------------------------------- entry point

def _run(nc, in_maps):
    global LAST_EXEC_NS
    trace = bool(os.environ.get("BASS_TRACE"))
    res = bass_utils.run_bass_kernel_spmd(
        nc, in_maps, list(range(N_CORES)), trace=trace,
        tmpdir=os.environ.get("BASS_TRACE_DIR") or None,
    )
    LAST_EXEC_NS = res.exec_time_ns
    return res


def kernel(x, W, d_bernoulli, bias):
    x = np.asarray(x, dtype=np.float32)
    W = np.asarray(W, dtype=np.float32)
    d_bernoulli = np.asarray(d_bernoulli, dtype=np.float32)
    bias = np.asarray(bias, dtype=np.float32)

    xT = np.ascontiguousarray((x * d_bernoulli[None, :]).T)

    if "fft" not in _CACHE:
        _CACHE["fft"] = _build_fft_nc()
    tb, mix, rd, beta = _fft_host_mats(W, bias)
    in_maps = []
    for c in range(N_CORES):
        xs = xT[:, c * NT:(c + 1) * NT]                    # (D, NT)
        # device layout: row j*128+p, col h*1024 + mt*512 + t
        xd = (xs.reshape(K, 2, 128, 2, 512)
              .transpose(0, 2, 3, 1, 4)
              .reshape(K * 128, 2 * NT)).astype(BF16_NP)
        in_maps.append({
            "xT": np.ascontiguousarray(xd),
            "tb": tb, "mix": mix, "rmat": rd, "beta": beta,
        })
    res = _run(_CACHE["fft"], in_maps)

    out = np.empty((B, D), dtype=np.float32)
    for c in range(N_CORES):
        out[c * NT:(c + 1) * NT, :] = res.results[c]["yT"].astype(np.float32).T
    return out
